# revision 1
# baseline (speedup 1.0000x reference)
"""Trainium2 Bass kernel for the SelfAttentionBlock problem (8 NeuronCores).

Sharding strategy:
  * MLP (q/k/v two-layer GELU blocks): data-parallel over rows — each core
    owns 256 tokens of each batch (512 rows total) and computes full-width
    q/k/v for those rows. No weight-partial sums, no all-reduce.
  * Attention: head-parallel — core c computes head c for both batches.
    Rows->heads redistribution is a single AllToAll per tensor (q, k, v).
  * k/v/attention outputs are gathered and reassembled on the host (the
    host-side concat is the "all-gather" of the final [N,S,D] outputs).

Layouts (all chosen so every matmul operand is produced in the layout the
TensorEngine wants — no on-device transposes anywhere):
  * x is shipped pre-transposed per core: xT [D, ROWS].
  * layer-1 output is h1T = gelu(W1^T xT + b1) [H, ROWS]  (lhsT=W1 natural).
  * q/k layer-2 produce qT/kT [D, ROWS] (lhsT=W2 natural, rhs=h1T).
  * v layer-2 produces v [ROWS, D] natural (lhsT=h1T, rhs=W2), bias via a
    K=1 ones-row matmul, outer gelu on ACT.
  * attention scores are computed transposed: scT [k, q] (lhsT=kT, rhs=qT),
    exp on ACT (scale=1/8 fused), causal mask via 0/1 band-mask multiply on
    DVE (only diagonal tiles), and the PV matmul consumes exp'd scores
    directly as the moving operand: oT [65, q] (lhsT=v_aug) where v_aug has
    a ones column so row 64 accumulates the softmax denominator.
  * fully-masked score tiles (above the causal diagonal) are never computed.
Matmuls run in float32r (full PE rate at fp32 storage, ~1e-4 matmul rel err).
"""

import ml_dtypes
import numpy as np

import concourse.bass as bass
import concourse.mybir as mybir
from concourse import bacc, tile
from concourse.bass_utils import run_bass_kernel_spmd

N_CORES = 8
N, S, D, H = 2, 2048, 512, 2048
HEADS = 8
Dh = D // HEADS            # 64
RPC = S // N_CORES         # 256 rows per core per batch
ROWS = N * RPC             # 512 rows per core
KT1 = D // 128             # 4 layer-1 contraction tiles
MT1 = H // 128             # 16 layer-1 out tiles == layer-2 contraction tiles
MT2 = D // 128             # 4 layer-2 out tiles
QC = S // 512              # 4 q-chunks per batch

F32 = mybir.dt.float32
F32R = mybir.dt.float32r
BF16 = mybir.dt.bfloat16
AF = mybir.ActivationFunctionType


def _build():
    nc = bacc.Bacc("TRN2", target_bir_lowering=False, debug=False,
                   num_devices=N_CORES)

    def din(name, shape):
        return nc.dram_tensor(name, shape, F32R, kind="ExternalInput")

    def dout(name, shape):
        return nc.dram_tensor(name, shape, F32R, kind="ExternalOutput")

    xT = nc.dram_tensor("xT", [D, ROWS], BF16, kind="ExternalInput")
    w1 = {t: nc.dram_tensor(f"w1{t}", [D, H], BF16, kind="ExternalInput")
          for t in "kqv"}
    w2 = {t: nc.dram_tensor(f"w2{t}", [H, D], BF16, kind="ExternalInput")
          for t in "kqv"}
    b1 = {t: din(f"b1{t}", [H]) for t in "kqv"}
    # b2 for k/q feeds DVE tensor_scalar_add, which requires plain float32
    b2 = {t: nc.dram_tensor(f"b2{t}", [D], F32 if t in "kq" else BF16,
                            kind="ExternalInput") for t in "kqv"}
    band_d = nc.dram_tensor("band", [128, 1024], BF16, kind="ExternalInput")
    ones_col_d = nc.dram_tensor("ones_col", [128, MT1], BF16, kind="ExternalInput")
    ones64f_d = nc.dram_tensor("ones64f", [1, 64], F32R, kind="ExternalInput")
    ones_row_d = nc.dram_tensor("ones_row", [1, 128], BF16, kind="ExternalInput")
    kT_out = nc.dram_tensor("kT_out", [D, ROWS], BF16, kind="ExternalOutput")
    v_out = nc.dram_tensor("v_out", [ROWS, D], BF16, kind="ExternalOutput")
    aT_out = dout("attn_outT", [N, Dh, S])

    with tile.TileContext(nc) as tc:
        with (
            tc.tile_pool(name="dram", bufs=1, space="DRAM") as dp,
            tc.tile_pool(name="cst", bufs=1) as cst,
            tc.tile_pool(name="w1p", bufs=2) as w1p,
            tc.tile_pool(name="w2p", bufs=8) as w2p,
            tc.tile_pool(name="h1p", bufs=18) as h1p,
            tc.tile_pool(name="l2p", bufs=4) as l2p,
            tc.tile_pool(name="att", bufs=2) as attp,
            tc.tile_pool(name="exp", bufs=12) as expp,
            tc.tile_pool(name="sm", bufs=4) as smp,
            tc.tile_pool(name="ps", bufs=4, space="PSUM") as psp,
            tc.tile_pool(name="pso", bufs=3, space="PSUM") as psop,
            tc.tile_pool(name="psb", bufs=1, space="PSUM") as psbp,
        ):
            send = {
                "k": dp.tile([D, ROWS], BF16, tag="send_k", name="send_k"),
                "q": dp.tile([D, ROWS], BF16, tag="send_q", name="send_q"),
                "v": dp.tile([N_CORES, ROWS, Dh], BF16, tag="send_v",
                             name="send_v"),
            }
            recv = {
                "k": dp.tile([D, ROWS], BF16, tag="recv_k", name="recv_k"),
                "q": dp.tile([D, ROWS], BF16, tag="recv_q", name="recv_q"),
                "v": dp.tile([N_CORES, ROWS, Dh], BF16, tag="recv_v",
                             name="recv_v"),
            }

            # ---- persistent tiles ----
            xt = cst.tile([128, KT1 * ROWS], BF16, tag="xt")
            nc.sync.dma_start(xt[:].rearrange("p (k r) -> p k r", k=KT1),
                              xT.ap().rearrange("(k p) r -> p k r", p=128))
            band_sb = cst.tile([128, 1024], BF16, tag="band")
            nc.sync.dma_start(band_sb[:], band_d[:])
            aux = cst.tile([1, 128 + D], BF16, tag="aux")
            nc.sync.dma_start(aux[:, 0:128], ones_row_d[:])
            ones128 = aux[:, 0:128]
            o64f = cst.tile([1, 64], F32R, tag="o64f")
            nc.sync.dma_start(o64f[:], ones64f_d[:])
            onescol = cst.tile([128, MT1], BF16, tag="onescol")
            nc.sync.dma_start(onescol[:], ones_col_d[:])
            b1_sb, b2qk_sb = {}, {}
            for t in "kqv":
                b1_sb[t] = cst.tile([128, MT1], F32R, tag=f"b1{t}", name=f"b1sb{t}")
                nc.sync.dma_start(b1_sb[t][:],
                                  b1[t].ap().rearrange("(m p) -> p m", p=128))
            for t in "kq":
                b2qk_sb[t] = cst.tile([128, MT2], F32, tag=f"b2{t}", name=f"b2sb{t}")
                nc.sync.dma_start(b2qk_sb[t][:],
                                  b2[t].ap().rearrange("(m p) -> p m", p=128))
            b2v_sb = aux[:, 128:128 + D]
            nc.sync.dma_start(b2v_sb, b2["v"].ap().rearrange("(a d) -> a d", a=1))

            def mlp(t, transposed):
                w1t = w1p.tile([128, KT1 * H], BF16, tag="w1", name=f"w1_{t}")
                nc.sync.dma_start(w1t[:].rearrange("p (k h) -> p k h", k=KT1),
                                  w1[t].ap().rearrange("(k p) h -> p k h", p=128))
                w2_t = []
                for g in range(MT1 // 4):
                    w = w2p.tile([128, 4 * D], BF16, tag="w2", name=f"w2_{t}{g}")
                    nc.sync.dma_start(
                        w[:].rearrange("p (k d) -> p k d", k=4),
                        w2[t][g * 512:(g + 1) * 512, :]
                        .rearrange("(k p) d -> p k d", p=128))
                    w2_t.append(w)
                h1_t = []
                for m in range(MT1):
                    pp = psp.tile([128, ROWS], F32, tag="ps", name=f"ps1_{t}{m}")
                    for kt in range(KT1):
                        nc.tensor.matmul(pp[:],
                                         w1t[:, kt * H + m * 128:
                                             kt * H + (m + 1) * 128],
                                         xt[:, kt * ROWS:(kt + 1) * ROWS],
                                         start=(kt == 0), stop=(kt == KT1 - 1))
                    h1 = h1p.tile([128, ROWS], BF16, tag="h1", name=f"h1_{t}{m}")
                    nc.scalar.activation(h1[:], pp[:], AF.Gelu_apprx_tanh,
                                         bias=b1_sb[t][:, m:m + 1])
                    h1_t.append(h1)
                if transposed:          # k, q: out = W2^T h1T + b2  [D, ROWS]
                    for m in range(MT2):
                        pp = psp.tile([128, ROWS], F32, tag="ps", name=f"ps2_{t}{m}")
                        for kt in range(MT1):
                            nc.tensor.matmul(pp[:],
                                             w2_t[kt // 4][:, (kt % 4) * D +
                                                           m * 128:(kt % 4) * D +
                                                           (m + 1) * 128],
                                             h1_t[kt][:],
                                             start=(kt == 0), stop=(kt == MT1 - 1))
                        ot = l2p.tile([128, ROWS], BF16, tag="l2", name=f"l2_{t}{m}")
                        with nc.allow_low_precision(reason="bf16 outputs"):
                            nc.vector.tensor_scalar_add(ot[:], pp[:],
                                                        b2qk_sb[t][:, m:m + 1])
                        nc.sync.dma_start(send[t][m * 128:(m + 1) * 128, :],
                                          ot[:])
                        if t == "k":
                            nc.sync.dma_start(kT_out[m * 128:(m + 1) * 128, :],
                                              ot[:])
                else:                   # v: out = gelu(h1 W2 + b2)  [ROWS, D]
                    for m in range(MT2):
                        pp = psp.tile([128, D], F32, tag="ps", name=f"ps2_{t}{m}")
                        for kt in range(MT1):
                            nc.tensor.matmul(pp[:],
                                             h1_t[kt][:, m * 128:(m + 1) * 128],
                                             w2_t[kt // 4][:, (kt % 4) * D:
                                                           (kt % 4 + 1) * D],
                                             start=(kt == 0), stop=False)
                        nc.tensor.matmul(pp[:], ones128, b2v_sb,
                                         start=False, stop=True)
                        ot = l2p.tile([128, D], BF16, tag="l2", name=f"l2_{t}{m}")
                        nc.scalar.activation(ot[:], pp[:], AF.Gelu_apprx_tanh)
                        nc.sync.dma_start(v_out[m * 128:(m + 1) * 128, :], ot[:])
                        nc.sync.dma_start(
                            send["v"][:, m * 128:(m + 1) * 128, :]
                            .rearrange("p r d -> r p d"),
                            ot[:].rearrange("r (p d) -> r p d", p=N_CORES))

            def a2a(t):
                nc.gpsimd.collective_compute(
                    "AllToAll", mybir.AluOpType.bypass,
                    replica_groups=[list(range(N_CORES))],
                    ins=[send[t].opt()], outs=[recv[t].opt()])

            mlp("k", True)
            a2a("k")
            mlp("q", True)
            a2a("q")
            mlp("v", False)
            a2a("v")

            # ---- attention: head c (this core), both batches interleaved ----
            qT_sb, kT_sb, vaug = {}, {}, {}
            for b in range(N):
                qT_sb[b] = attp.tile([Dh, S], BF16, tag="qT", name=f"qT{b}")
                kT_sb[b] = attp.tile([Dh, S], BF16, tag="kT", name=f"kT{b}")
                vaug[b] = attp.tile([128, MT1 * 65], BF16, tag="vaug",
                                    name=f"va{b}")
                nc.sync.dma_start(
                    qT_sb[b][:].rearrange("p (j r) -> p j r", j=N_CORES),
                    recv["q"][:, b * RPC:(b + 1) * RPC]
                    .rearrange("(j p) r -> p j r", p=Dh))
                nc.sync.dma_start(
                    kT_sb[b][:].rearrange("p (j r) -> p j r", j=N_CORES),
                    recv["k"][:, b * RPC:(b + 1) * RPC]
                    .rearrange("(j p) r -> p j r", p=Dh))
                nc.sync.dma_start(
                    vaug[b][:].rearrange("p (g c) -> p g c", c=65)[:, :, 64:65],
                    onescol[:].rearrange("p (g o) -> p g o", o=1))
                for h in range(2):
                    nc.gpsimd.dma_start(
                        vaug[b][:].rearrange("p (j h c) -> p j h c", j=N_CORES,
                                             h=2)[:, :, h, 0:64],
                        recv["v"][:, b * RPC + h * 128:b * RPC + (h + 1) * 128,
                                  :].rearrange("j p d -> p j d"))

            for qc in range(QC):
                q0 = qc * 512
                nk = 4 * qc + 4
                exps = {b: [None] * nk for b in range(N)}
                po = {}

                def scores(b, kt):
                    pp = psp.tile([128, 512], F32, tag="ps",
                                  name=f"sc{b}{qc}_{kt}")
                    nc.tensor.matmul(pp[:],
                                     kT_sb[b][:, kt * 128:(kt + 1) * 128],
                                     qT_sb[b][:, q0:q0 + 512],
                                     start=True, stop=True)
                    ex = expp.tile([128, 512], BF16, tag="exp",
                                   name=f"ex{b}{qc}_{kt}")
                    nc.scalar.activation(ex[:], pp[:], AF.Exp, scale=0.125)
                    o = kt * 128 - q0
                    if o >= 0:      # diagonal tile: 0/1 causal band mask
                        nc.vector.tensor_mul(ex[:], ex[:],
                                             band_sb[:, 512 - o:1024 - o])
                    exps[b][kt] = ex

                def pv(b, kt):
                    nc.tensor.matmul(po[b][:],
                                     vaug[b][:, kt * 65:(kt + 1) * 65],
                                     exps[b][kt][:],
                                     start=(kt == 0), stop=(kt == nk - 1))

                lag = 3
                for b in range(N):
                    po[b] = psop.tile([65, 512], F32, tag="pso",
                                      name=f"po{b}{qc}")
                    for kt in range(nk):
                        scores(b, kt)
                        if kt >= lag:
                            pv(b, kt - lag)
                    for kt in range(max(0, nk - lag), nk):
                        pv(b, kt)
                for b in range(N):
                    r_sb = smp.tile([1, 512], F32R, tag="r", name=f"r{b}{qc}")
                    with nc.allow_low_precision(reason="f32r is 32-bit"):
                        nc.vector.reciprocal(r_sb[:], po[b][64:65, :])
                    pb = psbp.tile([64, 512], F32, tag="psb", name=f"pb{b}{qc}")
                    nc.tensor.matmul(pb[:], o64f[:], r_sb[:],
                                     start=True, stop=True)
                    rb = smp.tile([64, 512], F32R, tag="rb", name=f"rb{b}{qc}")
                    nc.vector.tensor_copy(rb[:], pb[:])
                    oT = smp.tile([64, 512], F32R, tag="oT", name=f"oT{b}{qc}")
                    nc.vector.tensor_mul(oT[:], po[b][0:64, :], rb[:])
                    nc.sync.dma_start(aT_out[b, :, q0:q0 + 512], oT[:])

    nc.compile()
    return nc


_COMPILED = None


def _get_compiled():
    global _COMPILED
    if _COMPILED is None:
        _COMPILED = _build()
    return _COMPILED


def _band_mask():
    return (np.arange(1024, dtype=np.int32)[None, :]
            >= (np.arange(128, dtype=np.int32)[:, None] + 512)).astype(np.float32)


def _f32(a):
    return np.ascontiguousarray(np.asarray(a, dtype=np.float32))


def _bf16(a):
    return np.ascontiguousarray(np.asarray(a, dtype=np.float32)
                                .astype(ml_dtypes.bfloat16))


def _make_in_maps(x, qW1, qb1, qW2, qb2, kW1, kb1, kW2, kb2, vW1, vb1,
                  vW2, vb2):
    x = _f32(x)
    band = _band_mask()
    shared = {
        "w1q": _bf16(qW1), "w1k": _bf16(kW1), "w1v": _bf16(vW1),
        "w2q": _bf16(qW2), "w2k": _bf16(kW2), "w2v": _bf16(vW2),
        "b1q": _f32(qb1), "b1k": _f32(kb1), "b1v": _f32(vb1),
        "b2q": _f32(qb2), "b2k": _f32(kb2), "b2v": _bf16(vb2),
        "band": band.astype(ml_dtypes.bfloat16),
        "ones_col": np.ones((128, MT1), ml_dtypes.bfloat16),
        "ones_row": np.ones((1, 128), ml_dtypes.bfloat16),
        "ones64f": np.ones((1, 64), np.float32),
    }
    in_maps = []
    for c in range(N_CORES):
        xc = np.concatenate([x[b, c * RPC:(c + 1) * RPC, :] for b in range(N)], 0)
        im = dict(shared)
        im["xT"] = np.ascontiguousarray(xc.T).astype(ml_dtypes.bfloat16)
        in_maps.append(im)
    return in_maps


def _assemble(res):

    k_full = np.empty((N, S, D), np.float32)
    v_full = np.empty((N, S, D), np.float32)
    out_full = np.empty((N, S, D), np.float32)
    for j in range(N_CORES):
        kT_j = np.asarray(res[j]["kT_out"], np.float32)   # [D, ROWS]
        v_j = np.asarray(res[j]["v_out"], np.float32)     # [ROWS, D]
        aT_j = res[j]["attn_outT"]       # [N, Dh, S]
        for b in range(N):
            k_full[b, j * RPC:(j + 1) * RPC, :] = kT_j[:, b * RPC:(b + 1) * RPC].T
            v_full[b, j * RPC:(j + 1) * RPC, :] = v_j[b * RPC:(b + 1) * RPC, :]
            out_full[b, :, j * Dh:(j + 1) * Dh] = aT_j[b].T
    return k_full, v_full, out_full


def kernel(**inputs):
    nc = _get_compiled()
    in_maps = _make_in_maps(**inputs)
    res = run_bass_kernel_spmd(nc, in_maps, list(range(N_CORES))).results
    return _assemble(res)



# revision 12
# speedup vs baseline: 1.2372x; 1.2372x over previous
"""Trainium2 Bass kernel for the SelfAttentionBlock problem (8 NeuronCores).

Sharding (same as baseline): MLP data-parallel over rows (each core owns
512 rows), AllToAll per tensor to switch to head-parallel attention
(core c computes head c for both batches), host concat of outputs.

v2 rewrite, driven by the baseline trace (305us):
  * ~95 descriptor-heavy DMAs -> ~20 dense ones; all weights/consts are
    host-packed into the exact SBUF layouts so every load is one
    contiguous [128, X] transfer.
  * AllToAll triggers fire right after each MLP section (they were all
    serialized after the MLP in the baseline, costing a 74us PE hole).
  * recv-side layouts redesigned so the attention input loads are three
    dense DMAs (the gpsimd vaug gather with 128B descriptors is gone).
  * q/k for both batches live in partitions 0-63 / 64-127, so the two
    batches' score matmuls (K=64 each) run concurrently in the PE array
    (row tiling via base_partition).
  * exp is batched 2 score tiles per ACTIVATE ([128,1024] PSUM groups).
  * scores for the late q-chunks are emitted inside/after the v-MLP so
    the ACT exp stream starts early and the v AllToAll is hidden.
  * softmax denominator reciprocal: DVE reciprocal_approx_fast (~5x
    faster than nc.vector.reciprocal, which cost 3.3us per chunk).
  * attention output stored bf16 ([2,64,2048] per core), divided out of
    PSUM via a PE ones-broadcast matmul into the same PSUM bank.
"""

import ml_dtypes
import numpy as np

import concourse.bass as bass
import concourse.mybir as mybir
from concourse import bacc, tile
from concourse.bass_utils import run_bass_kernel_spmd

N_CORES = 8
N, S, D, H = 2, 2048, 512, 2048
HEADS = 8
Dh = D // HEADS            # 64
RPC = S // N_CORES         # 256 rows per core per batch
ROWS = N * RPC             # 512 rows per core
KT1 = D // 128             # 4 layer-1 contraction tiles
MT1 = H // 128             # 16 layer-1 out tiles == layer-2 contraction tiles
MT2 = D // 128             # 4 layer-2 out tiles
QC = S // 512              # 4 q-chunks per batch

F32 = mybir.dt.float32
BF16 = mybir.dt.bfloat16
AF = mybir.ActivationFunctionType

# const blob column offsets
CF_B1 = {"k": 0, "q": 16, "v": 32}
CF_B2 = {"k": 48, "q": 52}
CF_ONES64 = 56             # f32 ones, row 0 used as [1,64] bcast lhsT
CF_W = 120
CB_BAND = 0                # [128, 1024] causal band
CB_B2V = 1024              # row 0: v second-layer bias [512]
CB_W = 1536


def _build():
    nc = bacc.Bacc("TRN2", target_bir_lowering=False, debug=False,
                   num_devices=N_CORES)

    xt_d = nc.dram_tensor("xt", [128, KT1 * ROWS], BF16, kind="ExternalInput")
    w1_d = {t: nc.dram_tensor(f"w1{t}", [128, MT1 * KT1 * 128], BF16,
                              kind="ExternalInput") for t in "kqv"}
    w2_d = {t: nc.dram_tensor(f"w2{t}", [128, MT1 * D], BF16,
                              kind="ExternalInput") for t in "kqv"}
    cf32_d = nc.dram_tensor("cf32", [128, CF_W], F32, kind="ExternalInput")
    cbf_d = nc.dram_tensor("cbf", [128, CB_W], BF16, kind="ExternalInput")

    kT_out = nc.dram_tensor("kT_out", [HEADS, Dh, ROWS], BF16,
                            kind="ExternalOutput")
    v_out = nc.dram_tensor("v_out", [ROWS, D], BF16, kind="ExternalOutput")
    aT_out = nc.dram_tensor("attn_outT", [N, Dh, S], BF16,
                            kind="ExternalOutput")

    with tile.TileContext(nc) as tc:
        with (
            tc.tile_pool(name="dram", bufs=1, space="DRAM") as dp,
            tc.tile_pool(name="cst", bufs=1) as cst,
            tc.tile_pool(name="w1p", bufs=2) as w1p,
            tc.tile_pool(name="w2p", bufs=2) as w2p,
            tc.tile_pool(name="h1p", bufs=20) as h1p,
            tc.tile_pool(name="l2p", bufs=2) as l2p,
            tc.tile_pool(name="exp", bufs=34) as expp,
            tc.tile_pool(name="sm", bufs=3) as smp,
            tc.tile_pool(name="ps", bufs=2, space="PSUM") as psp,
            tc.tile_pool(name="po", bufs=4, space="PSUM") as pop,
        ):
            send = {
                "k": dp.tile([HEADS, Dh, ROWS], BF16, tag="send_k",
                             name="send_k"),
                "q": dp.tile([HEADS, Dh, ROWS], BF16, tag="send_q",
                             name="send_q"),
                "v": dp.tile([HEADS, 128, MT2, Dh], BF16, tag="send_v",
                             name="send_v"),
            }
            recv = {
                "k": dp.tile([HEADS, Dh, ROWS], BF16, tag="recv_k",
                             name="recv_k"),
                "q": dp.tile([HEADS, Dh, ROWS], BF16, tag="recv_q",
                             name="recv_q"),
                "v": dp.tile([HEADS, 128, MT2, Dh], BF16, tag="recv_v",
                             name="recv_v"),
            }

            # ---- persistent SBUF tiles / early DMAs ----
            xt = cst.tile([128, KT1 * ROWS], BF16, tag="xt")
            nc.sync.dma_start(xt[:], xt_d[:])
            w1sb = {"k": w1p.tile([128, MT1 * KT1 * 128], BF16, tag="w1",
                                  name="w1_k")}
            # split first weight load so L1-k can start sooner
            nc.sync.dma_start(w1sb["k"][:, 0:4096], w1_d["k"][:, 0:4096])
            nc.sync.dma_start(w1sb["k"][:, 4096:8192], w1_d["k"][:, 4096:8192])
            w2sb = {"k": w2p.tile([128, MT1 * D], BF16, tag="w2", name="w2_k")}
            nc.sync.dma_start(w2sb["k"][:], w2_d["k"][:])
            cf32 = cst.tile([128, CF_W], F32, tag="cf32")
            nc.sync.dma_start(cf32[:], cf32_d[:])
            cbf = cst.tile([128, CB_W], BF16, tag="cbf")
            nc.sync.dma_start(cbf[:], cbf_d[:])

            ones_row = cst.tile([1, 128], BF16, tag="ones_row")
            nc.vector.memset(ones_row[:], 1.0)
            # attention input tiles: both batches stacked on partitions for
            # q/k (b=0 -> partitions 0:64, b=1 -> 64:128)
            kT2 = cst.tile([128, HEADS * RPC], BF16, tag="kT2")
            qT2 = cst.tile([128, HEADS * RPC], BF16, tag="qT2")
            vaug = cst.tile([128, HEADS * N * 2 * 65], BF16, tag="vaug")
            with nc.allow_low_precision(reason="ones fill"):
                nc.vector.memset(
                    vaug[:].rearrange("p (g e) -> p g e", e=65)[:, :, 64:65],
                    1.0)
            oT_all = cst.tile([Dh, N * S], BF16, tag="oT")

            def mlp(t, after_l1_m0=None, after_l1=None, after_l2_mid=None):
                h1_t = []
                for m in range(MT1):
                    pp = psp.tile([128, 512], F32, tag="sc", name=f"p1{t}{m}")
                    for kt in range(KT1):
                        nc.tensor.matmul(
                            pp[:],
                            w1sb[t][:, m * 512 + kt * 128:m * 512 +
                                    (kt + 1) * 128],
                            xt[:, kt * ROWS:(kt + 1) * ROWS],
                            start=(kt == 0), stop=(kt == KT1 - 1))
                    h1 = h1p.tile([128, ROWS], BF16, tag="h1",
                                  name=f"h1{t}{m}")
                    nc.scalar.activation(h1[:], pp[:], AF.Gelu_apprx_tanh,
                                         bias=cf32[:, CF_B1[t] + m:
                                                   CF_B1[t] + m + 1])
                    h1_t.append(h1)
                    if m == 0 and after_l1_m0:
                        after_l1_m0()
                if after_l1:
                    after_l1()
                l2 = l2p.tile([128, MT2 * 512], BF16, tag="l2", name=f"l2{t}")
                for mo in range(MT2):
                    pp = psp.tile([128, 512], F32, tag="sc", name=f"p2{t}{mo}")
                    if t != "v":    # out = W2^T h1T + b2, transposed [D, ROWS]
                        for kt in range(MT1):
                            nc.tensor.matmul(
                                pp[:],
                                w2sb[t][:, kt * D + mo * 128:
                                        kt * D + (mo + 1) * 128],
                                h1_t[kt][:],
                                start=(kt == 0), stop=(kt == MT1 - 1))
                        with nc.allow_low_precision(reason="bf16 out"):
                            nc.vector.tensor_scalar_add(
                                l2[:, mo * 512:(mo + 1) * 512], pp[:],
                                cf32[:, CF_B2[t] + mo:CF_B2[t] + mo + 1])
                    else:           # v: out = gelu(h1 W2 + b2), natural
                        for kt in range(MT1):
                            nc.tensor.matmul(
                                pp[:],
                                h1_t[kt][:, mo * 128:(mo + 1) * 128],
                                w2sb[t][:, kt * D:(kt + 1) * D],
                                start=(kt == 0), stop=False)
                        nc.tensor.matmul(pp[:], ones_row[:],
                                         cbf[0:1, CB_B2V:CB_B2V + 512],
                                         start=False, stop=True)
                        nc.scalar.activation(l2[:, mo * 512:(mo + 1) * 512],
                                             pp[:], AF.Gelu_apprx_tanh)
                    if mo == 1 and after_l2_mid:
                        after_l2_mid()
                return l2

            def a2a(t):
                nc.gpsimd.collective_compute(
                    "AllToAll", mybir.AluOpType.bypass,
                    replica_groups=[list(range(N_CORES))],
                    ins=[send[t].opt()], outs=[recv[t].opt()])

            # ---------------- MLP k ----------------
            def load_qw():
                w1sb["q"] = w1p.tile([128, MT1 * KT1 * 128], BF16, tag="w1",
                                     name="w1_q")
                nc.sync.dma_start(w1sb["q"][:], w1_d["q"][:])
                w2sb["q"] = w2p.tile([128, MT1 * D], BF16, tag="w2",
                                     name="w2_q")
                nc.sync.dma_start(w2sb["q"][:], w2_d["q"][:])

            l2k = mlp("k", after_l1_m0=load_qw)
            # l2k layout [p, (m r)]; head j = (m=j//2, phalf=j%2)
            nc.sync.dma_start(
                send["k"][:].rearrange("(m m2) p r -> (m2 p) m r", m2=2),
                l2k[:].rearrange("p (m r) -> p m r", m=MT2))
            nc.sync.dma_start(
                kT_out.ap().rearrange("(m m2) p r -> (m2 p) m r", m2=2),
                l2k[:].rearrange("p (m r) -> p m r", m=MT2))
            a2a("k")

            # ---------------- MLP q ----------------
            def load_vw():
                w1sb["v"] = w1p.tile([128, MT1 * KT1 * 128], BF16, tag="w1",
                                     name="w1_v")
                nc.sync.dma_start(w1sb["v"][:], w1_d["v"][:])
                w2sb["v"] = w2p.tile([128, MT1 * D], BF16, tag="w2",
                                     name="w2_v")
                nc.sync.dma_start(w2sb["v"][:], w2_d["v"][:])

            l2q = mlp("q", after_l1_m0=load_vw)
            nc.sync.dma_start(
                send["q"][:].rearrange("(m m2) p r -> (m2 p) m r", m2=2),
                l2q[:].rearrange("p (m r) -> p m r", m=MT2))
            a2a("q")

            # recv-side loads for q/k on the Scalar HWDGE queue so they do
            # not block the Sync queue (which still has sends pending).
            def load_kq_recv():
                for b in range(N):
                    nc.scalar.dma_start(
                        kT2[b * 64:(b + 1) * 64, :]
                        .rearrange("p (j r) -> p j r", j=HEADS),
                        recv["k"][:, :, b * RPC:(b + 1) * RPC]
                        .rearrange("j p r -> p j r"))
                    nc.scalar.dma_start(
                        qT2[b * 64:(b + 1) * 64, :]
                        .rearrange("p (j r) -> p j r", j=HEADS),
                        recv["q"][:, :, b * RPC:(b + 1) * RPC]
                        .rearrange("j p r -> p j r"))

            # ---------------- attention helpers ----------------
            exps = {}

            def scores(b, qc):
                """Score matmuls + exp for chunk (b, qc); groups of 2 kt."""
                nk = 4 * qc + 4
                exps[(b, qc)] = []
                for g in range(nk // 2):
                    pp = psp.tile([128, 1024], F32, tag="sc",
                                  name=f"sc{b}{qc}{g}")
                    for h in range(2):
                        kt = 2 * g + h
                        nc.tensor.matmul(
                            pp[:, h * 512:(h + 1) * 512],
                            kT2[b * 64:(b + 1) * 64,
                                kt * 128:(kt + 1) * 128],
                            qT2[b * 64:(b + 1) * 64,
                                qc * 512:(qc + 1) * 512],
                            start=True, stop=True)
                    ex = expp.tile([128, 1024], BF16, tag="exp",
                                   name=f"ex{b}{qc}{g}")
                    nc.scalar.activation(ex[:], pp[:], AF.Exp, scale=0.125)
                    for h in range(2):
                        kt = 2 * g + h
                        o = kt * 128 - qc * 512
                        if o >= 0:   # diagonal tile: 0/1 causal band mask
                            with nc.allow_low_precision(reason="mask"):
                                nc.vector.tensor_mul(
                                    ex[:, h * 512:(h + 1) * 512],
                                    ex[:, h * 512:(h + 1) * 512],
                                    cbf[:, CB_BAND + 512 - o:
                                        CB_BAND + 1024 - o])
                    exps[(b, qc)].append(ex)

            po = {}

            def pv(b, qc):
                nk = 4 * qc + 4
                p = pop.tile([65, 512], F32, tag="po", name=f"po{b}{qc}")
                po[(b, qc)] = p
                for kt in range(nk):
                    g, h = kt // 2, kt % 2
                    j, h2 = kt // 2, kt % 2
                    gidx = j * 4 + b * 2 + h2
                    nc.tensor.matmul(
                        p[:],
                        vaug[:, gidx * 65:(gidx + 1) * 65],
                        exps[(b, qc)][g][:, h * 512:(h + 1) * 512],
                        start=(kt == 0), stop=(kt == nk - 1))

            def norm(b, qc):
                p = po[(b, qc)]
                d_sb = smp.tile([1, 512], F32, tag="d", name=f"d{b}{qc}")
                nc.vector.tensor_copy(d_sb[:], p[64:65, :])
                r = smp.tile([1, 512], F32, tag="r", name=f"r{b}{qc}")
                with nc.allow_low_precision(reason="approx recip"):
                    nc.vector.reciprocal_approx_fast(r[:], d_sb[:])
                rb = smp.tile([64, 512], F32, tag="rb", name=f"rb{b}{qc}")
                nc.gpsimd.partition_broadcast(rb[:], r[:], channels=64)
                with nc.allow_low_precision(reason="bf16"):
                    nc.vector.tensor_mul(
                        oT_all[:, b * S + qc * 512:b * S + (qc + 1) * 512],
                        p[0:64, :], rb[:])

            # ---------------- MLP v (+ early scores) ----------------
            l2v = mlp("v", after_l1_m0=load_kq_recv,
                      after_l1=lambda: scores(0, 3),
                      after_l2_mid=lambda: scores(1, 3))
            nc.sync.dma_start(
                v_out.ap().rearrange("(m p) d -> p m d", p=128),
                l2v[:].rearrange("p (m d) -> p m d", m=MT2))
            nc.sync.dma_start(
                send["v"][:].rearrange("c p m d -> p m c d"),
                l2v[:].rearrange("p (m c d) -> p m c d", c=HEADS, d=Dh))
            a2a("v")
            for b in range(N):
                for h in range(2):
                    nc.sync.dma_start(
                        vaug[:].rearrange("p (j b h e) -> p j b h e",
                                          j=HEADS, b=N, h=2)
                        [:, :, b, h, 0:64],
                        recv["v"][:, :, b * 2 + h, :]
                        .rearrange("j p d -> p j d"))

            # ---------------- attention main ----------------
            scores(0, 2)
            scores(1, 2)
            scores(0, 1)
            pv(0, 3)
            pv(1, 3)
            norm(0, 3)
            norm(1, 3)
            scores(1, 1)
            pv(0, 2)
            pv(1, 2)
            norm(0, 2)
            norm(1, 2)
            scores(0, 0)
            pv(0, 1)
            scores(1, 0)
            pv(1, 1)
            norm(0, 1)
            norm(1, 1)
            pv(0, 0)
            pv(1, 0)
            norm(0, 0)
            norm(1, 0)

            nc.sync.dma_start(
                aT_out.ap().rearrange("b p q -> p b q"),
                oT_all[:].rearrange("p (b q) -> p b q", b=N))

    nc.compile()
    return nc


_COMPILED = None


def _get_compiled():
    global _COMPILED
    if _COMPILED is None:
        _COMPILED = _build()
    return _COMPILED


def _band_mask():
    return (np.arange(1024, dtype=np.int32)[None, :]
            >= (np.arange(128, dtype=np.int32)[:, None] + 512)).astype(
                np.float32)


def _bf16(a):
    return np.ascontiguousarray(np.asarray(a, dtype=np.float32)
                                .astype(ml_dtypes.bfloat16))


def _pack_w1(w):            # [512, 2048] -> [128, (m kt 128)]
    w = np.asarray(w, np.float32)
    return _bf16(w.reshape(KT1, 128, MT1, 128).transpose(1, 2, 0, 3)
                 .reshape(128, MT1 * KT1 * 128))


def _pack_w2(w):            # [2048, 512] -> [128, (kt d)]
    w = np.asarray(w, np.float32)
    return _bf16(w.reshape(MT1, 128, D).transpose(1, 0, 2)
                 .reshape(128, MT1 * D))


def _make_in_maps(x, qW1, qb1, qW2, qb2, kW1, kb1, kW2, kb2, vW1, vb1,
                  vW2, vb2):
    x = np.asarray(x, np.float32)
    cf32 = np.zeros((128, CF_W), np.float32)
    for t, b1 in (("k", kb1), ("q", qb1), ("v", vb1)):
        cf32[:, CF_B1[t]:CF_B1[t] + 16] = np.asarray(b1, np.float32) \
            .reshape(16, 128).T
    for t, b2 in (("k", kb2), ("q", qb2)):
        cf32[:, CF_B2[t]:CF_B2[t] + 4] = np.asarray(b2, np.float32) \
            .reshape(4, 128).T
    cf32[:, CF_ONES64:CF_ONES64 + 64] = 1.0
    cbf = np.zeros((128, CB_W), np.float32)
    cbf[:, CB_BAND:CB_BAND + 1024] = _band_mask()
    cbf[0, CB_B2V:CB_B2V + 512] = np.asarray(vb2, np.float32)
    shared = {
        "w1q": _pack_w1(qW1), "w1k": _pack_w1(kW1), "w1v": _pack_w1(vW1),
        "w2q": _pack_w2(qW2), "w2k": _pack_w2(kW2), "w2v": _pack_w2(vW2),
        "cf32": cf32, "cbf": cbf.astype(ml_dtypes.bfloat16),
    }
    in_maps = []
    for c in range(N_CORES):
        xc = np.concatenate([x[b, c * RPC:(c + 1) * RPC, :]
                             for b in range(N)], 0)       # [ROWS, D]
        xT = np.ascontiguousarray(xc.T)                   # [D, ROWS]
        im = dict(shared)
        im["xt"] = _bf16(xT.reshape(KT1, 128, ROWS).transpose(1, 0, 2)
                         .reshape(128, KT1 * ROWS))
        in_maps.append(im)
    return in_maps


def _assemble(res):
    k_full = np.empty((N, S, D), np.float32)
    v_full = np.empty((N, S, D), np.float32)
    out_full = np.empty((N, S, D), np.float32)
    for j in range(N_CORES):
        kT_j = np.asarray(res[j]["kT_out"], np.float32)   # [8, 64, ROWS]
        v_j = np.asarray(res[j]["v_out"], np.float32)     # [ROWS, D]
        aT_j = np.asarray(res[j]["attn_outT"], np.float32)  # [N, Dh, S]
        # kT_j[h, p, b*256+rr] = k[b, j*RPC+rr, h*64+p]
        kk = kT_j.reshape(HEADS, Dh, N, RPC).transpose(2, 3, 0, 1) \
            .reshape(N, RPC, D)
        for b in range(N):
            k_full[b, j * RPC:(j + 1) * RPC, :] = kk[b]
            v_full[b, j * RPC:(j + 1) * RPC, :] = \
                v_j[b * RPC:(b + 1) * RPC, :]
            out_full[b, :, j * Dh:(j + 1) * Dh] = aT_j[b].T
    return k_full, v_full, out_full


def kernel(**inputs):
    nc = _get_compiled()
    in_maps = _make_in_maps(**inputs)
    res = run_bass_kernel_spmd(nc, in_maps, list(range(N_CORES))).results
    return _assemble(res)


# revision 16
# speedup vs baseline: 1.4770x; 1.1938x over previous
"""Trainium2 Bass kernel for the SelfAttentionBlock problem (8 NeuronCores).

Sharding (same as baseline): MLP data-parallel over rows (each core owns
512 rows), AllToAll per tensor to switch to head-parallel attention
(core c computes head c for both batches), host concat of outputs.

v2 rewrite, driven by the baseline trace (305us):
  * ~95 descriptor-heavy DMAs -> ~20 dense ones; all weights/consts are
    host-packed into the exact SBUF layouts so every load is one
    contiguous [128, X] transfer.
  * AllToAll triggers fire right after each MLP section (they were all
    serialized after the MLP in the baseline, costing a 74us PE hole).
  * recv-side layouts redesigned so the attention input loads are three
    dense DMAs (the gpsimd vaug gather with 128B descriptors is gone).
  * q/k for both batches live in partitions 0-63 / 64-127, so the two
    batches' score matmuls (K=64 each) run concurrently in the PE array
    (row tiling via base_partition).
  * exp is batched 2 score tiles per ACTIVATE ([128,1024] PSUM groups).
  * scores for the late q-chunks are emitted inside/after the v-MLP so
    the ACT exp stream starts early and the v AllToAll is hidden.
  * softmax denominator reciprocal: DVE reciprocal_approx_fast (~5x
    faster than nc.vector.reciprocal, which cost 3.3us per chunk).
  * attention output stored bf16 ([2,64,2048] per core), divided out of
    PSUM via a PE ones-broadcast matmul into the same PSUM bank.
"""

import ml_dtypes
import numpy as np

import concourse.bass as bass
import concourse.mybir as mybir
from concourse import bacc, tile
from concourse.bass_utils import run_bass_kernel_spmd

N_CORES = 8
N, S, D, H = 2, 2048, 512, 2048
HEADS = 8
Dh = D // HEADS            # 64
RPC = S // N_CORES         # 256 rows per core per batch
ROWS = N * RPC             # 512 rows per core
KT1 = D // 128             # 4 layer-1 contraction tiles
MT1 = H // 128             # 16 layer-1 out tiles == layer-2 contraction tiles
MT2 = D // 128             # 4 layer-2 out tiles
QC = S // 512              # 4 q-chunks per batch

F32 = mybir.dt.float32
BF16 = mybir.dt.bfloat16
AF = mybir.ActivationFunctionType

# const blob column offsets
CF_B1 = {"k": 0, "q": 16, "v": 32}
CF_B2 = {"k": 48, "q": 52}
CF_ONES64 = 56             # f32 ones, row 0 used as [1,64] bcast lhsT
CF_W = 120
CB_BAND = 0                # [128, 1024] causal band
CB_B2V = 1024              # row 0: v second-layer bias [512]
CB_W = 1536


def _build():
    nc = bacc.Bacc("TRN2", target_bir_lowering=False, debug=False,
                   num_devices=N_CORES)

    xt_d = nc.dram_tensor("xt", [128, KT1 * ROWS], BF16, kind="ExternalInput")
    w1_d = {t: nc.dram_tensor(f"w1{t}", [128, MT1 * KT1 * 128], BF16,
                              kind="ExternalInput") for t in "kqv"}
    w2_d = {t: nc.dram_tensor(f"w2{t}", [128, MT1 * D], BF16,
                              kind="ExternalInput") for t in "kqv"}
    cf32_d = nc.dram_tensor("cf32", [128, CF_W], F32, kind="ExternalInput")
    cbf_d = nc.dram_tensor("cbf", [128, CB_W], BF16, kind="ExternalInput")

    kT_out = nc.dram_tensor("kT_out", [HEADS, Dh, ROWS], BF16,
                            kind="ExternalOutput")
    v_out = nc.dram_tensor("v_out", [ROWS, D], BF16, kind="ExternalOutput")
    aT_out = nc.dram_tensor("attn_outT", [N, Dh, S], BF16,
                            kind="ExternalOutput")

    with tile.TileContext(nc) as tc:
        with (
            tc.tile_pool(name="dram", bufs=1, space="DRAM") as dp,
            tc.tile_pool(name="cst", bufs=1) as cst,
            tc.tile_pool(name="w1p", bufs=2) as w1p,
            tc.tile_pool(name="w2p", bufs=2) as w2p,
            tc.tile_pool(name="h1p", bufs=20) as h1p,
            tc.tile_pool(name="l2p", bufs=2) as l2p,
            tc.tile_pool(name="exp", bufs=34) as expp,
            tc.tile_pool(name="sm", bufs=3) as smp,
            tc.tile_pool(name="ps", bufs=2, space="PSUM") as psp,
            tc.tile_pool(name="po", bufs=4, space="PSUM") as pop,
        ):
            send = {
                "k": dp.tile([HEADS, Dh, ROWS], BF16, tag="send_k",
                             name="send_k"),
                "q": dp.tile([HEADS, Dh, ROWS], BF16, tag="send_q",
                             name="send_q"),
                "v": dp.tile([HEADS, 128, MT2, Dh], BF16, tag="send_v",
                             name="send_v"),
            }
            recv = {
                "k": dp.tile([HEADS, Dh, ROWS], BF16, tag="recv_k",
                             name="recv_k"),
                "q": dp.tile([HEADS, Dh, ROWS], BF16, tag="recv_q",
                             name="recv_q"),
                "v": dp.tile([HEADS, 128, MT2, Dh], BF16, tag="recv_v",
                             name="recv_v"),
            }

            # ---- persistent SBUF tiles / early DMAs ----
            xt = cst.tile([128, KT1 * ROWS], BF16, tag="xt")
            nc.sync.dma_start(xt[:], xt_d[:])
            w1sb = {"k": w1p.tile([128, MT1 * KT1 * 128], BF16, tag="w1",
                                  name="w1_k")}
            # split first weight load so L1-k can start sooner
            nc.sync.dma_start(w1sb["k"][:, 0:4096], w1_d["k"][:, 0:4096])
            nc.sync.dma_start(w1sb["k"][:, 4096:8192], w1_d["k"][:, 4096:8192])
            w2sb = {"k": w2p.tile([128, MT1 * D], BF16, tag="w2", name="w2_k")}
            nc.sync.dma_start(w2sb["k"][:], w2_d["k"][:])
            cf32 = cst.tile([128, CF_W], F32, tag="cf32")
            nc.sync.dma_start(cf32[:], cf32_d[:])
            cbf = cst.tile([128, CB_W], BF16, tag="cbf")
            nc.sync.dma_start(cbf[:], cbf_d[:])

            ones_row = cst.tile([1, 128], BF16, tag="ones_row")
            nc.vector.memset(ones_row[:], 1.0)
            # attention input tiles: both batches stacked on partitions for
            # q/k (b=0 -> partitions 0:64, b=1 -> 64:128)
            kT2 = cst.tile([128, HEADS * RPC], BF16, tag="kT2")
            qT2 = cst.tile([128, HEADS * RPC], BF16, tag="qT2")
            vaug = cst.tile([128, HEADS * N * 2 * 65], BF16, tag="vaug")
            with nc.allow_low_precision(reason="ones fill"):
                nc.vector.memset(
                    vaug[:].rearrange("p (g e) -> p g e", e=65)[:, :, 64:65],
                    1.0)
            oT_all = cst.tile([Dh, N * S], BF16, tag="oT")

            def mlp(t, after_l1_m0=None, after_l1=None, after_l2_mid=None):
                h1_t = []
                for m in range(MT1):
                    pp = psp.tile([128, 512], F32, tag="sc", name=f"p1{t}{m}")
                    for kt in range(KT1):
                        nc.tensor.matmul(
                            pp[:],
                            w1sb[t][:, m * 512 + kt * 128:m * 512 +
                                    (kt + 1) * 128],
                            xt[:, kt * ROWS:(kt + 1) * ROWS],
                            start=(kt == 0), stop=(kt == KT1 - 1))
                    h1 = h1p.tile([128, ROWS], BF16, tag="h1",
                                  name=f"h1{t}{m}")
                    nc.scalar.activation(h1[:], pp[:], AF.Gelu_apprx_tanh,
                                         bias=cf32[:, CF_B1[t] + m:
                                                   CF_B1[t] + m + 1])
                    h1_t.append(h1)
                    if m == 0 and after_l1_m0:
                        after_l1_m0()
                if after_l1:
                    after_l1()
                l2 = l2p.tile([128, MT2 * 512], BF16, tag="l2", name=f"l2{t}")
                for mo in range(MT2):
                    pp = psp.tile([128, 512], F32, tag="sc", name=f"p2{t}{mo}")
                    if t != "v":    # out = W2^T h1T + b2, transposed [D, ROWS]
                        for kt in range(MT1):
                            nc.tensor.matmul(
                                pp[:],
                                w2sb[t][:, kt * D + mo * 128:
                                        kt * D + (mo + 1) * 128],
                                h1_t[kt][:],
                                start=(kt == 0), stop=(kt == MT1 - 1))
                        with nc.allow_low_precision(reason="bf16 out"):
                            nc.vector.tensor_scalar_add(
                                l2[:, mo * 512:(mo + 1) * 512], pp[:],
                                cf32[:, CF_B2[t] + mo:CF_B2[t] + mo + 1])
                    else:           # v: out = gelu(h1 W2 + b2), natural
                        for kt in range(MT1):
                            nc.tensor.matmul(
                                pp[:],
                                h1_t[kt][:, mo * 128:(mo + 1) * 128],
                                w2sb[t][:, kt * D:(kt + 1) * D],
                                start=(kt == 0), stop=False)
                        nc.tensor.matmul(pp[:], ones_row[:],
                                         cbf[0:1, CB_B2V:CB_B2V + 512],
                                         start=False, stop=True)
                        nc.scalar.activation(l2[:, mo * 512:(mo + 1) * 512],
                                             pp[:], AF.Gelu_apprx_tanh)
                    if mo == 1 and after_l2_mid:
                        after_l2_mid()
                return l2

            def a2a(t):
                nc.gpsimd.collective_compute(
                    "AllToAll", mybir.AluOpType.bypass,
                    replica_groups=[list(range(N_CORES))],
                    ins=[send[t].opt()], outs=[recv[t].opt()])

            # ---------------- MLP k ----------------
            def load_qw():
                w1sb["q"] = w1p.tile([128, MT1 * KT1 * 128], BF16, tag="w1",
                                     name="w1_q")
                nc.sync.dma_start(w1sb["q"][:], w1_d["q"][:])
                w2sb["q"] = w2p.tile([128, MT1 * D], BF16, tag="w2",
                                     name="w2_q")
                nc.sync.dma_start(w2sb["q"][:], w2_d["q"][:])

            def load_vw():
                w1sb["v"] = w1p.tile([128, MT1 * KT1 * 128], BF16, tag="w1",
                                     name="w1_v")
                nc.sync.dma_start(w1sb["v"][:], w1_d["v"][:])
                w2sb["v"] = w2p.tile([128, MT1 * D], BF16, tag="w2",
                                     name="w2_v")
                nc.sync.dma_start(w2sb["v"][:], w2_d["v"][:])

            l2k = mlp("k", after_l1_m0=load_qw)
            # l2k layout [p, (m r)]; head j = (m=j//2, phalf=j%2)
            nc.sync.dma_start(
                send["k"][:].rearrange("(m m2) p r -> (m2 p) m r", m2=2),
                l2k[:].rearrange("p (m r) -> p m r", m=MT2))
            nc.sync.dma_start(
                kT_out.ap().rearrange("(m m2) p r -> (m2 p) m r", m2=2),
                l2k[:].rearrange("p (m r) -> p m r", m=MT2))
            load_vw()
            a2a("k")

            # ---------------- MLP q ----------------
            l2q = mlp("q")
            nc.sync.dma_start(
                send["q"][:].rearrange("(m m2) p r -> (m2 p) m r", m2=2),
                l2q[:].rearrange("p (m r) -> p m r", m=MT2))
            a2a("q")

            # recv-side loads; after the sends in the Sync queue so their
            # collective-completion waits cannot block a send (deadlock).
            for b in range(N):
                nc.sync.dma_start(
                    kT2[b * 64:(b + 1) * 64, :]
                    .rearrange("p (j r) -> p j r", j=HEADS),
                    recv["k"][:, :, b * RPC:(b + 1) * RPC]
                    .rearrange("j p r -> p j r"))
                nc.sync.dma_start(
                    qT2[b * 64:(b + 1) * 64, :]
                    .rearrange("p (j r) -> p j r", j=HEADS),
                    recv["q"][:, :, b * RPC:(b + 1) * RPC]
                    .rearrange("j p r -> p j r"))

            # ---------------- attention helpers ----------------
            exps = {}

            def scores(b, qc):
                """Score matmuls + exp for chunk (b, qc); groups of 2 kt."""
                nk = 4 * qc + 4
                exps[(b, qc)] = []
                for g in range(nk // 2):
                    pp = psp.tile([128, 1024], F32, tag="sc",
                                  name=f"sc{b}{qc}{g}")
                    for h in range(2):
                        kt = 2 * g + h
                        nc.tensor.matmul(
                            pp[:, h * 512:(h + 1) * 512],
                            kT2[b * 64:(b + 1) * 64,
                                kt * 128:(kt + 1) * 128],
                            qT2[b * 64:(b + 1) * 64,
                                qc * 512:(qc + 1) * 512],
                            start=True, stop=True)
                    ex = expp.tile([128, 1024], BF16, tag="exp",
                                   name=f"ex{b}{qc}{g}")
                    nc.scalar.activation(ex[:], pp[:], AF.Exp, scale=0.125)
                    for h in range(2):
                        kt = 2 * g + h
                        o = kt * 128 - qc * 512
                        if o >= 0:   # diagonal tile: 0/1 causal band mask
                            with nc.allow_low_precision(reason="mask"):
                                nc.vector.tensor_mul(
                                    ex[:, h * 512:(h + 1) * 512],
                                    ex[:, h * 512:(h + 1) * 512],
                                    cbf[:, CB_BAND + 512 - o:
                                        CB_BAND + 1024 - o])
                    exps[(b, qc)].append(ex)

            po = {}

            def pv(b, qc):
                nk = 4 * qc + 4
                p = pop.tile([65, 512], F32, tag="po", name=f"po{b}{qc}")
                po[(b, qc)] = p
                for kt in range(nk):
                    g, h = kt // 2, kt % 2
                    j, h2 = kt // 2, kt % 2
                    gidx = j * 4 + b * 2 + h2
                    nc.tensor.matmul(
                        p[:],
                        vaug[:, gidx * 65:(gidx + 1) * 65],
                        exps[(b, qc)][g][:, h * 512:(h + 1) * 512],
                        start=(kt == 0), stop=(kt == nk - 1))

            def norm(b, qc):
                p = po[(b, qc)]
                d_sb = smp.tile([1, 512], F32, tag="d", name=f"d{b}{qc}")
                nc.vector.tensor_copy(d_sb[:], p[64:65, :])
                r = smp.tile([1, 512], F32, tag="r", name=f"r{b}{qc}")
                with nc.allow_low_precision(reason="approx recip"):
                    nc.vector.reciprocal_approx_fast(r[:], d_sb[:])
                rb = smp.tile([64, 512], F32, tag="rb", name=f"rb{b}{qc}")
                nc.gpsimd.partition_broadcast(rb[:], r[:], channels=64)
                with nc.allow_low_precision(reason="bf16"):
                    nc.vector.tensor_mul(
                        oT_all[:, b * S + qc * 512:b * S + (qc + 1) * 512],
                        p[0:64, :], rb[:])

            # ---------------- MLP v ----------------
            l2v = mlp("v")
            nc.sync.dma_start(
                v_out.ap().rearrange("(m p) d -> p m d", p=128),
                l2v[:].rearrange("p (m d) -> p m d", m=MT2))
            nc.sync.dma_start(
                send["v"][:].rearrange("c p m d -> p m c d"),
                l2v[:].rearrange("p (m c d) -> p m c d", c=HEADS, d=Dh))
            a2a("v")
            for b in range(N):
                for h in range(2):
                    nc.sync.dma_start(
                        vaug[:].rearrange("p (j b h e) -> p j b h e",
                                          j=HEADS, b=N, h=2)
                        [:, :, b, h, 0:64],
                        recv["v"][:, :, b * 2 + h, :]
                        .rearrange("j p d -> p j d"))

            # ---------------- attention main ----------------
            scores(0, 3)
            scores(1, 3)
            scores(0, 2)
            scores(1, 2)
            scores(0, 1)
            pv(0, 3)
            pv(1, 3)
            norm(0, 3)
            norm(1, 3)
            scores(1, 1)
            pv(0, 2)
            pv(1, 2)
            norm(0, 2)
            norm(1, 2)
            scores(0, 0)
            pv(0, 1)
            scores(1, 0)
            pv(1, 1)
            norm(0, 1)
            norm(1, 1)
            pv(0, 0)
            pv(1, 0)
            norm(0, 0)
            norm(1, 0)

            nc.sync.dma_start(
                aT_out.ap().rearrange("b p q -> p b q"),
                oT_all[:].rearrange("p (b q) -> p b q", b=N))

    nc.compile()
    return nc


_COMPILED = None


def _get_compiled():
    global _COMPILED
    if _COMPILED is None:
        _COMPILED = _build()
    return _COMPILED


def _band_mask():
    return (np.arange(1024, dtype=np.int32)[None, :]
            >= (np.arange(128, dtype=np.int32)[:, None] + 512)).astype(
                np.float32)


def _bf16(a):
    return np.ascontiguousarray(np.asarray(a, dtype=np.float32)
                                .astype(ml_dtypes.bfloat16))


def _pack_w1(w):            # [512, 2048] -> [128, (m kt 128)]
    w = np.asarray(w, np.float32)
    return _bf16(w.reshape(KT1, 128, MT1, 128).transpose(1, 2, 0, 3)
                 .reshape(128, MT1 * KT1 * 128))


def _pack_w2(w):            # [2048, 512] -> [128, (kt d)]
    w = np.asarray(w, np.float32)
    return _bf16(w.reshape(MT1, 128, D).transpose(1, 0, 2)
                 .reshape(128, MT1 * D))


def _make_in_maps(x, qW1, qb1, qW2, qb2, kW1, kb1, kW2, kb2, vW1, vb1,
                  vW2, vb2):
    x = np.asarray(x, np.float32)
    cf32 = np.zeros((128, CF_W), np.float32)
    for t, b1 in (("k", kb1), ("q", qb1), ("v", vb1)):
        cf32[:, CF_B1[t]:CF_B1[t] + 16] = np.asarray(b1, np.float32) \
            .reshape(16, 128).T
    for t, b2 in (("k", kb2), ("q", qb2)):
        cf32[:, CF_B2[t]:CF_B2[t] + 4] = np.asarray(b2, np.float32) \
            .reshape(4, 128).T
    cf32[:, CF_ONES64:CF_ONES64 + 64] = 1.0
    cbf = np.zeros((128, CB_W), np.float32)
    cbf[:, CB_BAND:CB_BAND + 1024] = _band_mask()
    cbf[0, CB_B2V:CB_B2V + 512] = np.asarray(vb2, np.float32)
    shared = {
        "w1q": _pack_w1(qW1), "w1k": _pack_w1(kW1), "w1v": _pack_w1(vW1),
        "w2q": _pack_w2(qW2), "w2k": _pack_w2(kW2), "w2v": _pack_w2(vW2),
        "cf32": cf32, "cbf": cbf.astype(ml_dtypes.bfloat16),
    }
    in_maps = []
    for c in range(N_CORES):
        xc = np.concatenate([x[b, c * RPC:(c + 1) * RPC, :]
                             for b in range(N)], 0)       # [ROWS, D]
        xT = np.ascontiguousarray(xc.T)                   # [D, ROWS]
        im = dict(shared)
        im["xt"] = _bf16(xT.reshape(KT1, 128, ROWS).transpose(1, 0, 2)
                         .reshape(128, KT1 * ROWS))
        in_maps.append(im)
    return in_maps


def _assemble(res):
    k_full = np.empty((N, S, D), np.float32)
    v_full = np.empty((N, S, D), np.float32)
    out_full = np.empty((N, S, D), np.float32)
    for j in range(N_CORES):
        kT_j = np.asarray(res[j]["kT_out"], np.float32)   # [8, 64, ROWS]
        v_j = np.asarray(res[j]["v_out"], np.float32)     # [ROWS, D]
        aT_j = np.asarray(res[j]["attn_outT"], np.float32)  # [N, Dh, S]
        # kT_j[h, p, b*256+rr] = k[b, j*RPC+rr, h*64+p]
        kk = kT_j.reshape(HEADS, Dh, N, RPC).transpose(2, 3, 0, 1) \
            .reshape(N, RPC, D)
        for b in range(N):
            k_full[b, j * RPC:(j + 1) * RPC, :] = kk[b]
            v_full[b, j * RPC:(j + 1) * RPC, :] = \
                v_j[b * RPC:(b + 1) * RPC, :]
            out_full[b, :, j * Dh:(j + 1) * Dh] = aT_j[b].T
    return k_full, v_full, out_full


def kernel(**inputs):
    nc = _get_compiled()
    in_maps = _make_in_maps(**inputs)
    res = run_bass_kernel_spmd(nc, in_maps, list(range(N_CORES))).results
    return _assemble(res)
